# revision 19
# baseline (speedup 1.0000x reference)
"""Style-modulated Conv1d (StyleGAN-like) Trainium2 kernel.

Full-input contract: kernel(**inputs) takes the unsharded inputs and returns
the full (B, COUT, T) output. Internally the work is sharded over 8
NeuronCores: batch-groups of 4 samples x T-halves (4x2 grid), so each core
processes a [128, T/2] slab at full partition occupancy.

Math: with s = lrelu(style @ (fc_w * gain)^T + fc_b) and
d = rsqrt(sum_{cin,k} (w * s)^2 + eps), the modulated-demodulated conv
factors as   y = lrelu(conv(x, w_base * s[cin] * d[cout]) + nstr*noise + bias).
The style affine / modulation / demodulation touch only (B, COUT, CIN, K)
~50k values, so they run on the host in fp32; the device sees ready-made
block-diagonal fp16 conv taps (d folded in) and streams the 128 MiB conv.

Everything that scales with T runs in fp16: x and y move over HBM at half
the fp32 bytes (the memory roofline for this problem), and the PE streams
fp16 at 1 col/cycle (vs 4 cyc for fp32) so 3 taps fit under the DMA floor.
PSUM accumulation stays fp32. With bias=0 and noise_strength=0 (this
module's init) the epilogue is a single DVE op: y = max(v, 0.2*v) cast to
fp16 on write-out.
"""

import numpy as np

import concourse.bass as bass
import concourse.tile as tile
from concourse import bacc, mybir

F32 = mybir.dt.float32
F16 = mybir.dt.float16
BF16 = mybir.dt.bfloat16

B, CIN, COUT, T, WDIM, K = 16, 32, 32, 65536, 512, 3
ALPHA = 0.2
GAIN = float(1.0 / np.sqrt(np.float32(WDIM)))
EPS = 1e-8

N_CORES = 8
BG = 4          # samples per core (batch group)
TSPLIT = 2      # T split factor
T_LOC = T // TSPLIT

CH = 4096       # columns per DMA chunk (~1 MiB fp16 transfers)
HB = 2048       # columns per PSUM tile (4 banks of fp32)
MN = 512        # matmul free dim (one PSUM bank of fp32)


def chunk_plan(t_loc):
    """Small chunks at the edges (fast pipeline ramp/drain), 4096 middle."""
    head = [512, 512, 1024, 2048]
    tail = [2048, 1024, 512, 512]
    mid = (t_loc - sum(head) - sum(tail)) // CH
    plan = head + [CH] * mid + tail
    assert sum(plan) == t_loc
    return plan


def build_program(t_loc=T_LOC, with_noise=False, with_bias=False):
    """One-core Bass program; identical on all 8 cores (SPMD, data differs)."""
    mult = mybir.AluOpType.mult
    amax = mybir.AluOpType.max

    nc = bacc.Bacc("TRN2", target_bir_lowering=False, debug=False)
    xh = nc.dram_tensor("xh", [128, t_loc + 2], F16, kind="ExternalInput")
    wtap = nc.dram_tensor("wtap", [K, 128, 128], F16, kind="ExternalInput")
    if with_noise:
        nz = nc.dram_tensor("nz", [BG, t_loc], F16, kind="ExternalInput")
        wnz = nc.dram_tensor("wnz", [BG, 128], F16, kind="ExternalInput")
    if with_bias:
        bia = nc.dram_tensor("bia", [128, 1], F32, kind="ExternalInput")
    yh = nc.dram_tensor("yh", [128, t_loc], BF16, kind="ExternalOutput")

    with tile.TileContext(nc) as tc:
        with (
            tc.tile_pool(name="const", bufs=1) as cp,
            tc.tile_pool(name="xin", bufs=8) as xp,
            tc.tile_pool(name="nzin", bufs=3) as nzp,
            tc.tile_pool(name="outp", bufs=6) as outp,
            tc.tile_pool(name="zp", bufs=4) as zp,
            tc.tile_pool(name="ps", bufs=2, space="PSUM") as psp,
        ):
            # block-diagonal taps (demod pre-folded), one standalone tile per
            # tap so every ldweights AP starts at offset 0. The tap DMAs ride
            # the scalar engine's HWDGE ring so the first x-chunk load can
            # issue on sync's ring concurrently.
            wt = [cp.tile([128, 128], F16, name=f"wt{k}") for k in range(K)]
            for k in range(K):
                nc.scalar.dma_start(wt[k], wtap[k, :, :])
            if with_noise:
                wn = cp.tile([BG, 128], F16)
                nc.scalar.dma_start(wn, wnz[:, :])
            if with_bias:
                bia_sb = cp.tile([128, 1], F32)
                nc.scalar.dma_start(bia_sb, bia[:, :])

            col = 0
            for size in chunk_plan(t_loc):
                xt = xp.tile([128, CH + 2], F16, tag="xt", name="xt")[:, : size + 2]
                nc.sync.dma_start(xt, xh[:, col : col + size + 2])
                if with_noise:
                    nzt = nzp.tile([BG, CH], F16, tag="nzt", name="nzt")[:, :size]
                    nc.sync.dma_start(nzt, nz[:, col : col + size])
                for p0 in range(0, size, HB):
                    psz = min(HB, size - p0)
                    ps = psp.tile([128, HB], F32, tag="ps", name="ps")[:, :psz]
                    for k in range(K):
                        for g in range(psz // MN):
                            o = p0 + g * MN
                            nc.tensor.matmul(
                                ps[:, g * MN : (g + 1) * MN],
                                wt[k],
                                xt[:, o + k : o + k + MN],
                                start=(k == 0),
                                stop=(k == K - 1 and not with_noise),
                                skip_group_check=True,
                            )
                    if with_noise:
                        for g in range(psz // MN):
                            nc.tensor.matmul(
                                ps[:, g * MN : (g + 1) * MN],
                                wn,
                                nzt[:, p0 + g * MN : p0 + (g + 1) * MN],
                                start=False,
                                stop=True,
                                skip_group_check=True,
                            )
                    # PSUM -> SBUF move (+bias) on ACT, lrelu max on DVE
                    z = zp.tile([128, HB], BF16, tag="z", name="z")[:, :psz]
                    nc.scalar.activation(
                        z, ps,
                        mybir.ActivationFunctionType.Identity,
                        bias=(bia_sb[:, 0:1] if with_bias else 0.0),
                        scale=1.0,
                    )
                    ot = outp.tile([128, HB], BF16, tag="ot", name="ot")[:, :psz]
                    nc.vector.scalar_tensor_tensor(
                        ot, z, ALPHA, z, op0=mult, op1=amax
                    )
                    # outputs go out on the scalar engine's HWDGE ring so a
                    # pending store never head-of-line blocks the next x
                    # load issued on sync's ring
                    nc.scalar.dma_start(yh[:, col + p0 : col + p0 + psz], ot)
                col += size

    nc.compile()
    return nc


def _modulate(style, fc_weight, fc_bias, weight, noise_strength):
    """Host-side style affine + modulate + demodulate (tiny: ~50k values)."""
    s = style @ (fc_weight * GAIN).T + fc_bias          # [B, CIN]
    s = np.where(s >= 0, s, ALPHA * s).astype(np.float32)
    w = weight[None] * s[:, None, :, None]              # [B, COUT, CIN, K]
    d = 1.0 / np.sqrt(np.sum(w * w, axis=(2, 3)) + EPS)  # [B, COUT]
    wd = w * d[:, :, None, None]
    return wd


def shard_inputs(x, style, fc_weight, fc_bias, weight, bias, noise_strength,
                 noise, t_loc=T_LOC, force_noise=False, force_bias=False):
    """Build the 8 per-core input dicts (taps shared per batch group)."""
    x = np.asarray(x, dtype=np.float32)
    style = np.asarray(style, dtype=np.float32)
    fc_weight = np.asarray(fc_weight, dtype=np.float32)
    fc_bias = np.asarray(fc_bias, dtype=np.float32)
    weight = np.asarray(weight, dtype=np.float32)
    bias = np.asarray(bias, dtype=np.float32)
    noise_strength = np.asarray(noise_strength, dtype=np.float32)
    noise = np.asarray(noise, dtype=np.float32)

    b_, cin_, t_ = x.shape
    tsplit = t_ // t_loc
    with_noise = force_noise or bool(np.any(noise_strength != 0))
    with_bias = force_bias or bool(np.any(bias != 0))

    wd = _modulate(style, fc_weight, fc_bias, weight, noise_strength)
    # per batch group: block-diag [K, 128(cin), 128(cout)] fp16 taps
    ngrp = b_ // BG
    wtap_g = np.zeros((ngrp, K, 128, 128), np.float16)
    for g in range(ngrp):
        for b in range(BG):
            wtap_g[g, :, 32 * b : 32 * b + 32, 32 * b : 32 * b + 32] = (
                wd[BG * g + b].transpose(2, 1, 0)
            )
    if with_noise:
        wnz = np.zeros((ngrp, BG, 128), np.float16)
        for g in range(ngrp):
            for b in range(BG):
                wnz[g, b, 32 * b : 32 * b + 32] = noise_strength
        nz16 = noise.astype(np.float16)
    bia = bias.reshape(1, COUT, 1) if with_bias else None

    x16 = x.astype(np.float16)
    in_maps = []
    for c in range(ngrp * tsplit):
        g, h = divmod(c, tsplit)
        xs = x16[BG * g : BG * g + BG]  # [4, 32, T]
        xpad = np.zeros((BG, cin_, t_loc + 2), dtype=np.float16)
        lo = h * t_loc - 1
        hi = h * t_loc + t_loc + 1
        src_lo, src_hi = max(lo, 0), min(hi, t_)
        xpad[:, :, src_lo - lo : src_lo - lo + (src_hi - src_lo)] = (
            xs[:, :, src_lo:src_hi]
        )
        im = {
            "xh": np.ascontiguousarray(xpad.reshape(128, t_loc + 2)),
            "wtap": wtap_g[g],
        }
        if with_noise:
            im["nz"] = np.ascontiguousarray(
                nz16[BG * g : BG * g + BG, 0, h * t_loc : (h + 1) * t_loc]
            )
            im["wnz"] = wnz[g]
        if with_bias:
            im["bia"] = np.ascontiguousarray(
                np.tile(bias, BG).reshape(128, 1).astype(np.float32)
            )
        in_maps.append(im)
    return in_maps, with_noise, with_bias


def unshard_output(results, b_=B, t_loc=T_LOC, tsplit=TSPLIT):
    y = np.empty((b_, COUT, t_loc * tsplit), dtype=np.float32)
    for c, r in enumerate(results):
        g, h = divmod(c, tsplit)
        y[BG * g : BG * g + BG, :, h * t_loc : (h + 1) * t_loc] = (
            np.asarray(r["yh"]).astype(np.float32).reshape(BG, COUT, t_loc)
        )
    return y


_PROGRAM_CACHE = {}


def get_program(with_noise=False, with_bias=False):
    key = (with_noise, with_bias)
    if key not in _PROGRAM_CACHE:
        _PROGRAM_CACHE[key] = build_program(
            with_noise=with_noise, with_bias=with_bias
        )
    return _PROGRAM_CACHE[key]


def kernel(x, style, fc_weight, fc_bias, weight, bias, noise_strength, noise):
    from concourse import bass_utils

    in_maps, with_noise, with_bias = shard_inputs(
        x, style, fc_weight, fc_bias, weight, bias, noise_strength, noise
    )
    nc = get_program(with_noise=with_noise, with_bias=with_bias)
    res = bass_utils.run_bass_kernel_spmd(nc, in_maps, core_ids=list(range(N_CORES)))
    return unshard_output(res.results)


# revision 21
# speedup vs baseline: 1.0086x; 1.0086x over previous
"""Style-modulated Conv1d (StyleGAN-like) Trainium2 kernel.

Full-input contract: kernel(**inputs) takes the unsharded inputs and returns
the full (B, COUT, T) output. Internally the work is sharded over 8
NeuronCores: batch-groups of 4 samples x T-halves (4x2 grid), so each core
processes a [128, T/2] slab at full partition occupancy.

Math: with s = lrelu(style @ (fc_w * gain)^T + fc_b) and
d = rsqrt(sum_{cin,k} (w * s)^2 + eps), the modulated-demodulated conv
factors as   y = lrelu(conv(x, w_base * s[cin] * d[cout]) + nstr*noise + bias).
The style affine / modulation / demodulation touch only (B, COUT, CIN, K)
~50k values, so they run on the host in fp32; the device sees ready-made
block-diagonal fp16 conv taps (d folded in) and streams the 128 MiB conv.

Everything that scales with T runs in fp16: x and y move over HBM at half
the fp32 bytes (the memory roofline for this problem), and the PE streams
fp16 at 1 col/cycle (vs 4 cyc for fp32) so 3 taps fit under the DMA floor.
PSUM accumulation stays fp32. With bias=0 and noise_strength=0 (this
module's init) the epilogue is a single DVE op: y = max(v, 0.2*v) cast to
fp16 on write-out.
"""

import numpy as np

import concourse.bass as bass
import concourse.tile as tile
from concourse import bacc, mybir

F32 = mybir.dt.float32
F16 = mybir.dt.float16
BF16 = mybir.dt.bfloat16

B, CIN, COUT, T, WDIM, K = 16, 32, 32, 65536, 512, 3
ALPHA = 0.2
GAIN = float(1.0 / np.sqrt(np.float32(WDIM)))
EPS = 1e-8

N_CORES = 8
BG = 4          # samples per core (batch group)
TSPLIT = 2      # T split factor
T_LOC = T // TSPLIT

CH = 4096       # columns per DMA chunk (~1 MiB fp16 transfers)
HB = 2048       # columns per PSUM tile (4 banks of fp32)
MN = 512        # matmul free dim (one PSUM bank of fp32)


def chunk_plan(t_loc):
    """Small chunks at the edges (fast pipeline ramp/drain), 4096 middle."""
    head = [1024, 1024, 2048]
    tail = [2048, 1024, 1024]
    mid = (t_loc - sum(head) - sum(tail)) // CH
    plan = head + [CH] * mid + tail
    assert sum(plan) == t_loc
    return plan


def build_program(t_loc=T_LOC, with_noise=False, with_bias=False):
    """One-core Bass program; identical on all 8 cores (SPMD, data differs)."""
    mult = mybir.AluOpType.mult
    amax = mybir.AluOpType.max

    nc = bacc.Bacc("TRN2", target_bir_lowering=False, debug=False)
    xh = nc.dram_tensor("xh", [128, t_loc + 2], F16, kind="ExternalInput")
    wtap = nc.dram_tensor("wtap", [K, 128, 128], F16, kind="ExternalInput")
    if with_noise:
        nz = nc.dram_tensor("nz", [BG, t_loc], F16, kind="ExternalInput")
        wnz = nc.dram_tensor("wnz", [BG, 128], F16, kind="ExternalInput")
    if with_bias:
        bia = nc.dram_tensor("bia", [128, 1], F32, kind="ExternalInput")
    yh = nc.dram_tensor("yh", [128, t_loc], BF16, kind="ExternalOutput")

    with tile.TileContext(nc) as tc:
        with (
            tc.tile_pool(name="const", bufs=1) as cp,
            tc.tile_pool(name="xin", bufs=8) as xp,
            tc.tile_pool(name="nzin", bufs=3) as nzp,
            tc.tile_pool(name="outp", bufs=6) as outp,
            tc.tile_pool(name="zp", bufs=4) as zp,
            tc.tile_pool(name="ps", bufs=2, space="PSUM") as psp,
        ):
            # block-diagonal taps (demod pre-folded), one standalone tile per
            # tap so every ldweights AP starts at offset 0. The tap DMAs ride
            # the scalar engine's HWDGE ring so the first x-chunk load can
            # issue on sync's ring concurrently.
            wt = [cp.tile([128, 128], F16, name=f"wt{k}") for k in range(K)]
            for k in range(K):
                nc.scalar.dma_start(wt[k], wtap[k, :, :])
            if with_noise:
                wn = cp.tile([BG, 128], F16)
                nc.scalar.dma_start(wn, wnz[:, :])
            if with_bias:
                bia_sb = cp.tile([128, 1], F32)
                nc.scalar.dma_start(bia_sb, bia[:, :])

            col = 0
            for size in chunk_plan(t_loc):
                xt = xp.tile([128, CH + 2], F16, tag="xt", name="xt")[:, : size + 2]
                nc.sync.dma_start(xt, xh[:, col : col + size + 2])
                if with_noise:
                    nzt = nzp.tile([BG, CH], F16, tag="nzt", name="nzt")[:, :size]
                    nc.sync.dma_start(nzt, nz[:, col : col + size])
                for p0 in range(0, size, HB):
                    psz = min(HB, size - p0)
                    ps = psp.tile([128, HB], F32, tag="ps", name="ps")[:, :psz]
                    for k in range(K):
                        for g in range(psz // MN):
                            o = p0 + g * MN
                            nc.tensor.matmul(
                                ps[:, g * MN : (g + 1) * MN],
                                wt[k],
                                xt[:, o + k : o + k + MN],
                                start=(k == 0),
                                stop=(k == K - 1 and not with_noise),
                                skip_group_check=True,
                            )
                    if with_noise:
                        for g in range(psz // MN):
                            nc.tensor.matmul(
                                ps[:, g * MN : (g + 1) * MN],
                                wn,
                                nzt[:, p0 + g * MN : p0 + (g + 1) * MN],
                                start=False,
                                stop=True,
                                skip_group_check=True,
                            )
                    # PSUM -> SBUF move (+bias) on ACT, lrelu max on DVE
                    z = zp.tile([128, HB], BF16, tag="z", name="z")[:, :psz]
                    nc.scalar.activation(
                        z, ps,
                        mybir.ActivationFunctionType.Identity,
                        bias=(bia_sb[:, 0:1] if with_bias else 0.0),
                        scale=1.0,
                    )
                    ot = outp.tile([128, HB], BF16, tag="ot", name="ot")[:, :psz]
                    nc.vector.scalar_tensor_tensor(
                        ot, z, ALPHA, z, op0=mult, op1=amax
                    )
                    # stores issue from the otherwise-idle GPSIMD engine so
                    # they never head-of-line block x loads (sync ring) nor
                    # serialize the ACT pipeline (scalar ring)
                    nc.gpsimd.dma_start(yh[:, col + p0 : col + p0 + psz], ot)
                col += size

    nc.compile()
    return nc


def _modulate(style, fc_weight, fc_bias, weight, noise_strength):
    """Host-side style affine + modulate + demodulate (tiny: ~50k values)."""
    s = style @ (fc_weight * GAIN).T + fc_bias          # [B, CIN]
    s = np.where(s >= 0, s, ALPHA * s).astype(np.float32)
    w = weight[None] * s[:, None, :, None]              # [B, COUT, CIN, K]
    d = 1.0 / np.sqrt(np.sum(w * w, axis=(2, 3)) + EPS)  # [B, COUT]
    wd = w * d[:, :, None, None]
    return wd


def shard_inputs(x, style, fc_weight, fc_bias, weight, bias, noise_strength,
                 noise, t_loc=T_LOC, force_noise=False, force_bias=False):
    """Build the 8 per-core input dicts (taps shared per batch group)."""
    x = np.asarray(x, dtype=np.float32)
    style = np.asarray(style, dtype=np.float32)
    fc_weight = np.asarray(fc_weight, dtype=np.float32)
    fc_bias = np.asarray(fc_bias, dtype=np.float32)
    weight = np.asarray(weight, dtype=np.float32)
    bias = np.asarray(bias, dtype=np.float32)
    noise_strength = np.asarray(noise_strength, dtype=np.float32)
    noise = np.asarray(noise, dtype=np.float32)

    b_, cin_, t_ = x.shape
    tsplit = t_ // t_loc
    with_noise = force_noise or bool(np.any(noise_strength != 0))
    with_bias = force_bias or bool(np.any(bias != 0))

    wd = _modulate(style, fc_weight, fc_bias, weight, noise_strength)
    # per batch group: block-diag [K, 128(cin), 128(cout)] fp16 taps
    ngrp = b_ // BG
    wtap_g = np.zeros((ngrp, K, 128, 128), np.float16)
    for g in range(ngrp):
        for b in range(BG):
            wtap_g[g, :, 32 * b : 32 * b + 32, 32 * b : 32 * b + 32] = (
                wd[BG * g + b].transpose(2, 1, 0)
            )
    if with_noise:
        wnz = np.zeros((ngrp, BG, 128), np.float16)
        for g in range(ngrp):
            for b in range(BG):
                wnz[g, b, 32 * b : 32 * b + 32] = noise_strength
        nz16 = noise.astype(np.float16)
    bia = bias.reshape(1, COUT, 1) if with_bias else None

    x16 = x.astype(np.float16)
    in_maps = []
    for c in range(ngrp * tsplit):
        g, h = divmod(c, tsplit)
        xs = x16[BG * g : BG * g + BG]  # [4, 32, T]
        xpad = np.zeros((BG, cin_, t_loc + 2), dtype=np.float16)
        lo = h * t_loc - 1
        hi = h * t_loc + t_loc + 1
        src_lo, src_hi = max(lo, 0), min(hi, t_)
        xpad[:, :, src_lo - lo : src_lo - lo + (src_hi - src_lo)] = (
            xs[:, :, src_lo:src_hi]
        )
        im = {
            "xh": np.ascontiguousarray(xpad.reshape(128, t_loc + 2)),
            "wtap": wtap_g[g],
        }
        if with_noise:
            im["nz"] = np.ascontiguousarray(
                nz16[BG * g : BG * g + BG, 0, h * t_loc : (h + 1) * t_loc]
            )
            im["wnz"] = wnz[g]
        if with_bias:
            im["bia"] = np.ascontiguousarray(
                np.tile(bias, BG).reshape(128, 1).astype(np.float32)
            )
        in_maps.append(im)
    return in_maps, with_noise, with_bias


def unshard_output(results, b_=B, t_loc=T_LOC, tsplit=TSPLIT):
    y = np.empty((b_, COUT, t_loc * tsplit), dtype=np.float32)
    for c, r in enumerate(results):
        g, h = divmod(c, tsplit)
        y[BG * g : BG * g + BG, :, h * t_loc : (h + 1) * t_loc] = (
            np.asarray(r["yh"]).astype(np.float32).reshape(BG, COUT, t_loc)
        )
    return y


_PROGRAM_CACHE = {}


def get_program(with_noise=False, with_bias=False):
    key = (with_noise, with_bias)
    if key not in _PROGRAM_CACHE:
        _PROGRAM_CACHE[key] = build_program(
            with_noise=with_noise, with_bias=with_bias
        )
    return _PROGRAM_CACHE[key]


def kernel(x, style, fc_weight, fc_bias, weight, bias, noise_strength, noise):
    from concourse import bass_utils

    in_maps, with_noise, with_bias = shard_inputs(
        x, style, fc_weight, fc_bias, weight, bias, noise_strength, noise
    )
    nc = get_program(with_noise=with_noise, with_bias=with_bias)
    res = bass_utils.run_bass_kernel_spmd(nc, in_maps, core_ids=list(range(N_CORES)))
    return unshard_output(res.results)


# revision 24
# speedup vs baseline: 1.0421x; 1.0332x over previous
"""Style-modulated Conv1d (StyleGAN-like) Trainium2 kernel.

Full-input contract: kernel(**inputs) takes the unsharded inputs and returns
the full (B, COUT, T) output. Internally the work is sharded over 8
NeuronCores: batch-groups of 4 samples x T-halves (4x2 grid), so each core
processes a [128, T/2] slab at full partition occupancy.

Math: with s = lrelu(style @ (fc_w * gain)^T + fc_b) and
d = rsqrt(sum_{cin,k} (w * s)^2 + eps), the modulated-demodulated conv
factors as   y = lrelu(conv(x, w_base * s[cin] * d[cout]) + nstr*noise + bias).
The style affine / modulation / demodulation touch only (B, COUT, CIN, K)
~50k values, so they run on the host in fp32; the device sees ready-made
block-diagonal fp16 conv taps (d folded in) and streams the 128 MiB conv.

Everything that scales with T runs in fp16: x and y move over HBM at half
the fp32 bytes (the memory roofline for this problem), and the PE streams
fp16 at 1 col/cycle (vs 4 cyc for fp32) so 3 taps fit under the DMA floor.
PSUM accumulation stays fp32. With bias=0 and noise_strength=0 (this
module's init) the epilogue is a single DVE op: y = max(v, 0.2*v) cast to
fp16 on write-out.
"""

import ml_dtypes
import numpy as np

import concourse.bass as bass
import concourse.tile as tile
from concourse import bacc, mybir

F32 = mybir.dt.float32
F16 = mybir.dt.float16
BF16 = mybir.dt.bfloat16

B, CIN, COUT, T, WDIM, K = 16, 32, 32, 65536, 512, 3
ALPHA = 0.2
GAIN = float(1.0 / np.sqrt(np.float32(WDIM)))
EPS = 1e-8

N_CORES = 8
BG = 4          # samples per core (batch group)
TSPLIT = 2      # T split factor
T_LOC = T // TSPLIT

CH = 4096       # columns per DMA chunk (~1 MiB fp16 transfers)
HB = 2048       # columns per PSUM tile (4 banks of fp32)
MN = 512        # matmul free dim (one PSUM bank of fp32)


def chunk_plan(t_loc):
    """Small chunks at the edges (fast pipeline ramp/drain), 4096 middle."""
    head = [2048, 2048]
    tail = [2048, 1024, 512, 512]
    mid = (t_loc - sum(head) - sum(tail)) // CH
    plan = head + [CH] * mid + tail
    assert sum(plan) == t_loc
    return plan


def build_program(t_loc=T_LOC, with_noise=False, with_bias=False):
    """One-core Bass program; identical on all 8 cores (SPMD, data differs)."""
    mult = mybir.AluOpType.mult
    amax = mybir.AluOpType.max

    nc = bacc.Bacc("TRN2", target_bir_lowering=False, debug=False)
    xh = nc.dram_tensor("xh", [128, t_loc + 2], F16, kind="ExternalInput")
    wtap = nc.dram_tensor("wtap", [K, 128, 128], F16, kind="ExternalInput")
    if with_noise:
        nz = nc.dram_tensor("nz", [BG, t_loc], F16, kind="ExternalInput")
        wnz = nc.dram_tensor("wnz", [BG, 128], F16, kind="ExternalInput")
    if with_bias:
        bia = nc.dram_tensor("bia", [128, 1], F32, kind="ExternalInput")
    yh = nc.dram_tensor("yh", [128, t_loc], F16, kind="ExternalOutput")

    with tile.TileContext(nc) as tc:
        with (
            tc.tile_pool(name="const", bufs=1) as cp,
            tc.tile_pool(name="xin", bufs=8) as xp,
            tc.tile_pool(name="nzin", bufs=3) as nzp,
            tc.tile_pool(name="outp", bufs=6) as outp,
            tc.tile_pool(name="zp", bufs=4) as zp,
            tc.tile_pool(name="ps", bufs=2, space="PSUM") as psp,
        ):
            # block-diagonal taps (demod pre-folded), one standalone tile per
            # tap so every ldweights AP starts at offset 0. The tap DMAs ride
            # the scalar engine's HWDGE ring so the first x-chunk load can
            # issue on sync's ring concurrently.
            wt = [cp.tile([128, 128], F16, name=f"wt{k}") for k in range(K)]
            for k in range(K):
                nc.scalar.dma_start(wt[k], wtap[k, :, :])
            if with_noise:
                wn = cp.tile([BG, 128], F16)
                nc.scalar.dma_start(wn, wnz[:, :])
            if with_bias:
                bia_sb = cp.tile([128, 1], F32)
                nc.scalar.dma_start(bia_sb, bia[:, :])

            col = 0
            plan = chunk_plan(t_loc)
            for ci, size in enumerate(plan):
                last_chunk = ci >= len(plan) - 2
                xt = xp.tile([128, CH + 2], F16, tag="xt", name="xt")[:, : size + 2]
                nc.sync.dma_start(xt, xh[:, col : col + size + 2])
                if with_noise:
                    nzt = nzp.tile([BG, CH], F16, tag="nzt", name="nzt")[:, :size]
                    nc.sync.dma_start(nzt, nz[:, col : col + size])
                for p0 in range(0, size, HB):
                    psz = min(HB, size - p0)
                    mn = MN if psz % MN == 0 else psz
                    ps = psp.tile([128, HB], F32, tag="ps", name="ps")[:, :psz]
                    for k in range(K):
                        for g in range(psz // mn):
                            o = p0 + g * mn
                            nc.tensor.matmul(
                                ps[:, g * mn : (g + 1) * mn],
                                wt[k],
                                xt[:, o + k : o + k + mn],
                                start=(k == 0),
                                stop=(k == K - 1 and not with_noise),
                                skip_group_check=True,
                            )
                    if with_noise:
                        for g in range(psz // mn):
                            nc.tensor.matmul(
                                ps[:, g * mn : (g + 1) * mn],
                                wn,
                                nzt[:, p0 + g * mn : p0 + (g + 1) * mn],
                                start=False,
                                stop=True,
                                skip_group_check=True,
                            )
                    # PSUM -> SBUF move (+bias) on ACT, lrelu max on DVE
                    z = zp.tile([128, HB], F16, tag="z", name="z")[:, :psz]
                    nc.scalar.activation(
                        z, ps,
                        mybir.ActivationFunctionType.Identity,
                        bias=(bia_sb[:, 0:1] if with_bias else 0.0),
                        scale=1.0,
                    )
                    ot = outp.tile([128, HB], F16, tag="ot", name="ot")[:, :psz]
                    nc.vector.scalar_tensor_tensor(
                        ot, z, ALPHA, z, op0=mult, op1=amax
                    )
                    # stores issue from the otherwise-idle GPSIMD engine
                    # so they never head-of-line block x loads (sync ring)
                    # nor serialize the ACT pipeline (scalar ring). The last
                    # chunk uses sync's HWDGE ring (idle by then, faster
                    # completion for the end-of-kernel barrier).
                    eng = nc.sync if last_chunk else nc.gpsimd
                    eng.dma_start(yh[:, col + p0 : col + p0 + psz], ot)
                col += size

    nc.compile()
    return nc


def _modulate(style, fc_weight, fc_bias, weight, noise_strength):
    """Host-side style affine + modulate + demodulate (tiny: ~50k values)."""
    s = style @ (fc_weight * GAIN).T + fc_bias          # [B, CIN]
    s = np.where(s >= 0, s, ALPHA * s).astype(np.float32)
    w = weight[None] * s[:, None, :, None]              # [B, COUT, CIN, K]
    d = 1.0 / np.sqrt(np.sum(w * w, axis=(2, 3)) + EPS)  # [B, COUT]
    wd = w * d[:, :, None, None]
    return wd


def shard_inputs(x, style, fc_weight, fc_bias, weight, bias, noise_strength,
                 noise, t_loc=T_LOC, force_noise=False, force_bias=False):
    """Build the 8 per-core input dicts (taps shared per batch group)."""
    x = np.asarray(x, dtype=np.float32)
    style = np.asarray(style, dtype=np.float32)
    fc_weight = np.asarray(fc_weight, dtype=np.float32)
    fc_bias = np.asarray(fc_bias, dtype=np.float32)
    weight = np.asarray(weight, dtype=np.float32)
    bias = np.asarray(bias, dtype=np.float32)
    noise_strength = np.asarray(noise_strength, dtype=np.float32)
    noise = np.asarray(noise, dtype=np.float32)

    b_, cin_, t_ = x.shape
    tsplit = t_ // t_loc
    with_noise = force_noise or bool(np.any(noise_strength != 0))
    with_bias = force_bias or bool(np.any(bias != 0))

    wd = _modulate(style, fc_weight, fc_bias, weight, noise_strength)
    # per batch group: block-diag [K, 128(cin), 128(cout)] bf16 taps
    ngrp = b_ // BG
    wtap_g = np.zeros((ngrp, K, 128, 128), np.float16)
    for g in range(ngrp):
        for b in range(BG):
            wtap_g[g, :, 32 * b : 32 * b + 32, 32 * b : 32 * b + 32] = (
                wd[BG * g + b].transpose(2, 1, 0)
            )
    if with_noise:
        wnz = np.zeros((ngrp, BG, 128), np.float16)
        for g in range(ngrp):
            for b in range(BG):
                wnz[g, b, 32 * b : 32 * b + 32] = noise_strength
        nz16 = noise.astype(np.float16)
    bia = bias.reshape(1, COUT, 1) if with_bias else None

    x16 = x.astype(np.float16)
    in_maps = []
    for c in range(ngrp * tsplit):
        g, h = divmod(c, tsplit)
        xs = x16[BG * g : BG * g + BG]  # [4, 32, T]
        xpad = np.zeros((BG, cin_, t_loc + 2), dtype=np.float16)
        lo = h * t_loc - 1
        hi = h * t_loc + t_loc + 1
        src_lo, src_hi = max(lo, 0), min(hi, t_)
        xpad[:, :, src_lo - lo : src_lo - lo + (src_hi - src_lo)] = (
            xs[:, :, src_lo:src_hi]
        )
        im = {
            "xh": np.ascontiguousarray(xpad.reshape(128, t_loc + 2)),
            "wtap": wtap_g[g],
        }
        if with_noise:
            im["nz"] = np.ascontiguousarray(
                nz16[BG * g : BG * g + BG, 0, h * t_loc : (h + 1) * t_loc]
            )
            im["wnz"] = wnz[g]
        if with_bias:
            im["bia"] = np.ascontiguousarray(
                np.tile(bias, BG).reshape(128, 1).astype(np.float32)
            )
        in_maps.append(im)
    return in_maps, with_noise, with_bias


def unshard_output(results, b_=B, t_loc=T_LOC, tsplit=TSPLIT):
    y = np.empty((b_, COUT, t_loc * tsplit), dtype=np.float32)
    for c, r in enumerate(results):
        g, h = divmod(c, tsplit)
        y[BG * g : BG * g + BG, :, h * t_loc : (h + 1) * t_loc] = (
            np.asarray(r["yh"]).astype(np.float32).reshape(BG, COUT, t_loc)
        )
    return y


_PROGRAM_CACHE = {}


def get_program(with_noise=False, with_bias=False):
    key = (with_noise, with_bias)
    if key not in _PROGRAM_CACHE:
        _PROGRAM_CACHE[key] = build_program(
            with_noise=with_noise, with_bias=with_bias
        )
    return _PROGRAM_CACHE[key]


def kernel(x, style, fc_weight, fc_bias, weight, bias, noise_strength, noise):
    from concourse import bass_utils

    in_maps, with_noise, with_bias = shard_inputs(
        x, style, fc_weight, fc_bias, weight, bias, noise_strength, noise
    )
    nc = get_program(with_noise=with_noise, with_bias=with_bias)
    res = bass_utils.run_bass_kernel_spmd(nc, in_maps, core_ids=list(range(N_CORES)))
    return unshard_output(res.results)
